# revision 40
# baseline (speedup 1.0000x reference)
"""Trainium2 Bass kernel for seq2seq GRU — table-lookup fp8 version.

B=4096, S=T=16, V=128, E=256, H=512. Pure data parallel over 8 cores
(batch sharded 512 words/core, weights replicated; forward only, so no
collectives needed).

Key idea: V=128 fits the PE contraction dim, so every x-path matmul
(x @ Wx) collapses into a 128-row table lookup:
  * candidate xh preact: host-gathered EXACT fp16 tiles, DMA'd per step
    (removes the 3-term hi/lo fp8 xh matmuls entirely);
  * z/r gate x-preact: one-hot DoubleRow matmul against an fp8 hi+lo
    table pair (exact to ~1e-3 rel, same PE cost as a direct fp8 x path).

Numerics (rel err ~1.2e-2 vs the 2e-2 budget):
  * state carried fp16 at scale 1; fp8e4m3 'hi' copy feeds the PE;
  * Wh-zr fp8 single; Wh-hh fp8 hi+lo pair at the same x32 scale;
  * out_W fp16; preacts land x32 in PSUM, ACT applies 1/32.

Structure: encoder = 4 independent recurrence chains (2 dirs x 2 batch
halves, W=256); decoder = 4 quarter chains (W=128) + a batched logits
matmul per step. One shared PSUM ring ([P,4,256] x4 = all 8 banks)
spans both phases so no pool-release barrier sits between them.

Engine split per GRU step (tuned against TimelineSim):
  PE    zr-h + one-hot zr-x + hh DoubleRow matmuls (plus dec logits)
  ACT   sigmoid(r), sigmoid(z), tanh(w)
  DVE   v=r*hh (PSUM read), most of w/d/m/T' (fp16 all-SBUF 2x mode),
        dec logit evac
  Pool  small k-tile shares of w/T' (w 1/4 enc, 1/2 dec; m 1/4 dec)
        and the fp16->fp8 hi state copy
"""

import numpy as np
import ml_dtypes

import concourse.bass as bass
import concourse.bacc as bacc
import concourse.mybir as mybir
from concourse.tile import TileContext
from concourse.bass_utils import run_bass_kernel_spmd

F32 = mybir.dt.float32
F16 = mybir.dt.float16
FP8 = mybir.dt.float8e4
AF = mybir.ActivationFunctionType
OP = mybir.AluOpType
DR = mybir.MatmulPerfMode.DoubleRow
NPF8 = ml_dtypes.float8_e4m3fn

P = 128
NCORES = 8
B, S, T = 4096, 16, 16
V, E, H = 128, 256, 512
BC = B // NCORES          # 512 words per core
HB = BC // 2              # encoder chain width
QB = BC // 4              # decoder chain width
KH = H // P               # 4
BOW = 1

SW = 32.0                 # preact scale in PSUM
ISW = float(1.0 / SW)

_ENC_CFG = {"w_kpool": 1, "d_kpool": 0, "m_kpool": 0, "t_kpool": 1,
            "hi_kpool": 4}
_DEC_CFG = {"w_kpool": 2, "d_kpool": 0, "m_kpool": 1, "t_kpool": 0,
            "hi_kpool": 4, "ps_width": HB}
_DEFAULT_CFG = {"enc": _ENC_CFG, "dec": _DEC_CFG, "ohp_bufs": 6,
                "xhp_bufs": 6, "enc_ps_bufs": 4, "dec_ps_bufs": 6,
                "gA_bufs": 3, "gB_bufs": 3, "st_bufs": 2, "std_bufs": 2}

_DEC_W_FETCH_T = [-1]
_LAST_STEP_INTERLEAVE = [False]
_PEND_DEPTH = [1]

LAST_RESULT = None
_CACHED_NC = None


def _gru_step(nc, ps, gpA, gpB, W, wzr, whh, zrx, oh, xh, st_prev, st_new,
              first, last=False, cfg=None):
    """One GRU step, transposed layout, width W (256 enc / 128 dec).

    oh: one-hot tile slice [P, 2, W] fp8 (k-tiles hi/lo of the zr table);
    xh: exact candidate x-preact slice [P, KH, W] fp16 (x32 scale).
    st_* = {"T16": fp16 state, "hi": fp8 copy}; on the first step only the
    z gate is computed (h=0 -> r unused, h' = (1-z)*c via sigma(-x)).
    """
    PSW = cfg.get("ps_width", W)

    def ptile(nm):
        t = ps.tile([P, KH, PSW], F32, tag="ps", name=nm)
        return t if PSW == W else t[:, :, 0:W]

    if first:
        zt = ptile("z_ps")
        for jj in range(KH):
            col = jj * P
            nc.tensor.matmul(zt[:, jj, :], zrx[:, :, col:col + P], oh,
                             start=True, stop=True, perf_mode=DR)
        zp16 = gpA.tile([P, KH, W], F16, tag="z16", name="zp16")
        nc.scalar.activation(zp16[:], zt[:], AF.Sigmoid, scale=-ISW)
        c16 = gpA.tile([P, KH, W], F16, tag="c16", name="c16")
        nc.scalar.activation(c16[:], xh, AF.Tanh, scale=ISW)
        nc.vector.tensor_tensor(st_new["T16"][:], zp16[:], c16[:], OP.mult)
        if not last:
            nc.gpsimd.tensor_copy(out=st_new["hi"][:], in_=st_new["T16"][:])
        return

    hi = st_prev["hi"]

    def zr_cols(t, jj, col):
        nc.tensor.matmul(t[:, jj, :], wzr[:, 0:2, col:col + P],
                         hi[:, 0:2, :], start=True, stop=False,
                         perf_mode=DR)
        nc.tensor.matmul(t[:, jj, :], wzr[:, 2:4, col:col + P],
                         hi[:, 2:4, :], start=False, stop=False,
                         perf_mode=DR)
        nc.tensor.matmul(t[:, jj, :], zrx[:, :, col:col + P], oh,
                         start=False, stop=True, perf_mode=DR)

    rt = ptile("r_ps")
    for jj in range(KH):
        zr_cols(rt, jj, H + jj * P)
    thh = ptile("hh_ps")
    for jj in range(KH):
        col = jj * P
        nc.tensor.matmul(thh[:, jj, :], whh[:, 0:2, col:col + P],
                         hi[:, 0:2, :], start=True, stop=False, perf_mode=DR)
        nc.tensor.matmul(thh[:, jj, :], whh[:, 2:4, col:col + P],
                         hi[:, 2:4, :], start=False, stop=False, perf_mode=DR)
        nc.tensor.matmul(thh[:, jj, :], whh[:, 4:6, col:col + P],
                         hi[:, 0:2, :], start=False, stop=False, perf_mode=DR)
        nc.tensor.matmul(thh[:, jj, :], whh[:, 6:8, col:col + P],
                         hi[:, 2:4, :], start=False, stop=True, perf_mode=DR)

    zt = ptile("z_ps")
    for jj in range(KH):
        zr_cols(zt, jj, jj * P)
    r16 = gpA.tile([P, KH, W], F16, tag="r16", name="r16")
    nc.scalar.activation(r16[:], rt[:], AF.Sigmoid, scale=ISW)
    z16 = gpA.tile([P, KH, W], F16, tag="z16", name="z16")
    nc.scalar.activation(z16[:], zt[:], AF.Sigmoid, scale=ISW)

    def split_tt(out, a, b, op, kpool):
        # first kpool k-tiles on Pool, rest on DVE
        if kpool > 0:
            nc.gpsimd.tensor_tensor(out[:, 0:kpool, :], a[:, 0:kpool, :],
                                    b[:, 0:kpool, :], op)
        if kpool < KH:
            nc.vector.tensor_tensor(out[:, kpool:KH, :], a[:, kpool:KH, :],
                                    b[:, kpool:KH, :], op)

    v16 = gpB.tile([P, KH, W], F16, tag="v16", name="v16")
    nc.vector.tensor_tensor(v16[:], r16[:], thh[:], OP.mult)
    if cfg.get("w_acc"):
        # accumulate v into the DMA'd xh tile slice in place (4x-mode
        # tensor_scalar with AddAccumulate) -> no separate w tile/op pair
        bi = nc.vector.tensor_scalar(xh, v16[:], 0.0, 1.0,
                                     OP.add, OP.mult)
        bi.ins.acc = "AddAccumulate"
        w16 = xh
    else:
        w16t = gpB.tile([P, KH, W], F16, tag="w16", name="w16")
        split_tt(w16t, v16, xh, OP.add, cfg["w_kpool"])
        w16 = w16t[:]
    c16 = gpA.tile([P, KH, W], F16, tag="c16", name="c16")
    nc.scalar.activation(c16[:], w16, AF.Tanh, scale=ISW)

    d16 = gpB.tile([P, KH, W], F16, tag="d16", name="d16")
    split_tt(d16, st_prev["T16"], c16, OP.subtract, cfg["d_kpool"])
    if cfg.get("acc_blend"):
        # T' = c, then T' += z*d in one accumulating TT (no m tile)
        nc.vector.tensor_copy(out=st_new["T16"][:], in_=c16[:])
        bi = nc.vector.tensor_tensor(st_new["T16"][:], z16[:], d16[:],
                                     OP.mult)
        bi.ins.acc = "AddAccumulate"
    else:
        m16 = gpB.tile([P, KH, W], F16, tag="m16", name="m16")
        split_tt(m16, z16, d16, OP.mult, cfg["m_kpool"])
        split_tt(st_new["T16"], c16, m16, OP.add, cfg["t_kpool"])
    if not last:
        kp = cfg["hi_kpool"]
        if kp > 0:
            nc.gpsimd.tensor_copy(out=st_new["hi"][:, 0:kp, :],
                                  in_=st_new["T16"][:, 0:kp, :])
        if kp < KH:
            nc.vector.tensor_copy(out=st_new["hi"][:, kp:KH, :],
                                  in_=st_new["T16"][:, kp:KH, :])


def _build_bass(cfg=None):
    cfg = dict(_DEFAULT_CFG, **(cfg or {}))
    nc = bacc.Bacc("TRN2", target_bir_lowering=False, debug=False,
                   enable_asserts=False, num_devices=NCORES)

    def din(name, shape, dt=FP8):
        return nc.dram_tensor(name, list(shape), dt, kind="ExternalInput")

    wzr_d = {c: din("wzr_" + c, (P, KH, 2 * H)) for c in "fbd"}
    whh_d = {c: din("whh_" + c, (P, 2 * KH, H)) for c in "fbd"}
    zrx_d = {c: din("zrx_" + c, (P, 2, 2 * H)) for c in "fbd"}
    ohe_d = din("ohe", (S, P, 2, BC))
    ohd_d = din("ohd", (T, P, 2, BC))
    xhe_d = din("xhe", (S, P, KH, BC), F16)
    xhb_d = din("xhb", (S, P, KH, BC), F16)
    xhd_d = din("xhd", (T, P, KH, BC), F16)
    ow_d = din("ow", (P, KH, V), F16)
    outb_d = din("outb", (P, 1), F32)
    out_d = nc.dram_tensor("out", [T, P, BC], F32, kind="ExternalOutput")

    with TileContext(nc) as tc:
        with tc.tile_pool(name="wpool", bufs=1) as wp, \
             tc.tile_pool(name="stenc", bufs=1) as stenc, \
             tc.tile_pool(name="ohp", bufs=cfg["ohp_bufs"]) as ohp, \
             tc.tile_pool(name="xhp", bufs=cfg["xhp_bufs"]) as xhp:

            # ---- weight tiles; DMA order = first-use order ----
            zrx = {c: wp.tile([P, 2, 2 * H], FP8, tag="zrx" + c,
                              name="zrx_" + c) for c in "fbd"}
            wzr = {c: wp.tile([P, KH, 2 * H], FP8, tag="wzr" + c,
                              name="wzr_" + c) for c in "fbd"}
            whh = {c: wp.tile([P, 2 * KH, H], FP8, tag="whh" + c,
                              name="whh_" + c) for c in "fbd"}
            ow_sb = wp.tile([P, KH, V], F16, tag="ow", name="ow_sb")
            outb_sb = wp.tile([P, 1], F32, tag="outb", name="outb_sb")

            # first steps need only the tables + first oh/xh tiles
            pre_oh, pre_xh = {}, {}
            def fetch_step(t, xsrc):
                oht = ohp.tile([P, 2, BC], FP8, tag="oh", name="oh_t")
                nc.sync.dma_start(out=oht[:], in_=ohe_d[t])
                pre_oh[t] = oht
                xht = xhp.tile([P, KH, BC], F16, tag="xh", name="xh_t")
                nc.sync.dma_start(out=xht[:], in_=xsrc[t])
                pre_xh[t] = xht
            nc.sync.dma_start(out=zrx["f"][:], in_=zrx_d["f"][:])
            fetch_step(0, xhe_d)
            nc.sync.dma_start(out=zrx["b"][:], in_=zrx_d["b"][:])
            fetch_step(S - 1, xhb_d)
            for c in "fb":
                nc.sync.dma_start(out=wzr[c][:], in_=wzr_d[c][:])
                nc.sync.dma_start(out=whh[c][:], in_=whh_d[c][:])
            fetch_step(1, xhe_d)
            fetch_step(S - 2, xhb_d)

            def fetch_dec_weights():
                for c in "d":
                    nc.sync.dma_start(out=zrx[c][:], in_=zrx_d[c][:])
                    nc.sync.dma_start(out=wzr[c][:], in_=wzr_d[c][:])
                    nc.sync.dma_start(out=whh[c][:], in_=whh_d[c][:])
                nc.sync.dma_start(out=ow_sb[:], in_=ow_d[:])
                nc.sync.dma_start(out=outb_sb[:], in_=outb_d[:])

            if _DEC_W_FETCH_T[0] < 0:
                fetch_dec_weights()


            # ---- shared pools: one PSUM ring for enc+dec (no release
            # barrier between phases); SBUF pools coexist ----
            with tc.tile_pool(name="ps", bufs=cfg["enc_ps_bufs"], space="PSUM") as ps, \
                 tc.tile_pool(name="st", bufs=cfg["st_bufs"]) as st, \
                 tc.tile_pool(name="gA", bufs=cfg["gA_bufs"]) as gpA, \
                 tc.tile_pool(name="gB", bufs=cfg["gB_bufs"]) as gpB, \
                 tc.tile_pool(name="std", bufs=cfg["std_bufs"]) as std, \
                 tc.tile_pool(name="g2A", bufs=4) as gp2A, \
                 tc.tile_pool(name="g2B", bufs=4) as gp2B, \
                 tc.tile_pool(name="lo", bufs=4) as lop:

                fin = {c + str(hf_): stenc.tile([P, KH, HB], F16,
                                                tag="fin" + c + str(hf_),
                                                name="fin" + c + str(hf_))
                       for c in "fb" for hf_ in range(2)}

                def new_state(tag):
                    return {
                        "T16": st.tile([P, KH, HB], F16, tag=tag + "T",
                                       name=tag + "T"),
                        "hi": st.tile([P, KH, HB], FP8, tag=tag + "h",
                                      name=tag + "h"),
                    }

                stt = {}
                for t in range(S):
                    if t == _DEC_W_FETCH_T[0]:
                        fetch_dec_weights()
                    if t < 2:
                        ohf, ohb = pre_oh[t], pre_oh[S - 1 - t]
                        xhf, xhb = pre_xh[t], pre_xh[S - 1 - t]
                    else:
                        ohf = ohp.tile([P, 2, BC], FP8, tag="oh", name="oh_f")
                        nc.sync.dma_start(out=ohf[:], in_=ohe_d[t])
                        xhf = xhp.tile([P, KH, BC], F16, tag="xh", name="xh_f")
                        nc.sync.dma_start(out=xhf[:], in_=xhe_d[t])
                        ohb = ohp.tile([P, 2, BC], FP8, tag="oh", name="oh_b")
                        nc.sync.dma_start(out=ohb[:], in_=ohe_d[S - 1 - t])
                        xhb = xhp.tile([P, KH, BC], F16, tag="xh", name="xh_b")
                        nc.sync.dma_start(out=xhb[:], in_=xhb_d[S - 1 - t])
                    if t == S - 1 and _LAST_STEP_INTERLEAVE[0]:
                        corder = [("f", ohf, xhf, 0), ("b", ohb, xhb, 0),
                                  ("f", ohf, xhf, 1), ("b", ohb, xhb, 1)]
                    else:
                        corder = [(c, o, x, hf_)
                                  for c, o, x in (("f", ohf, xhf),
                                                  ("b", ohb, xhb))
                                  for hf_ in range(2)]
                    for c, oht, xht, hf_ in corder:
                        if True:
                            key = c + str(hf_)
                            sl = slice(hf_ * HB, (hf_ + 1) * HB)
                            if t == S - 1:
                                sn = {"T16": fin[key], "hi": None}
                            else:
                                sn = new_state(key)
                            _gru_step(nc, ps, gpA, gpB, HB, wzr[c],
                                      whh[c], zrx[c], oht[:, :, sl],
                                      xht[:, :, sl], stt.get(key), sn,
                                      first=(t == 0), last=(t == S - 1),
                                      cfg=cfg["enc"])
                            stt[key] = sn

                # ---- decoder: 4 batch-quarter chains, W = QB ----
                def dec_state(tag):
                    return {
                        "T16": std.tile([P, KH, QB], F16, tag=tag + "T",
                                        name=tag + "T"),
                        "hi": std.tile([P, KH, QB], FP8, tag=tag + "h",
                                       name=tag + "h"),
                    }

                hs = {}
                for q in range(4):
                    s0 = dec_state("i%d" % q)
                    hf_, off = q // 2, (q % 2) * QB
                    nc.vector.tensor_tensor(
                        s0["T16"][:], fin["f%d" % hf_][:, :, off:off + QB],
                        fin["b%d" % hf_][:, :, off:off + QB], OP.add)
                    nc.gpsimd.tensor_copy(out=s0["hi"][:], in_=s0["T16"][:])
                    hs[q] = s0

                def emit_logits(states, t):
                    lpf = ps.tile([P, KH, HB], F32, tag="ps", name="lp")
                    lp = lpf[:, :, 0:QB]
                    for q in range(4):
                        for k in range(KH):
                            nc.tensor.matmul(lp[:, q, :], ow_sb[:, k, :],
                                             states[q]["T16"][:, k, :],
                                             start=(k == 0),
                                             stop=(k == KH - 1))
                    losb = lop.tile([P, 4, QB], F32, tag="lo", name="losb")
                    nc.vector.tensor_scalar_add(losb[:], lp[:],
                                                outb_sb[:, 0:1])
                    nc.sync.dma_start(out=out_d[t], in_=losb[:])

                pend = []
                for t in range(T):
                    oht = ohp.tile([P, 2, BC], FP8, tag="oh", name="ohd_t")
                    nc.sync.dma_start(out=oht[:], in_=ohd_d[t])
                    xht = xhp.tile([P, KH, BC], F16, tag="xh", name="xhd_t")
                    nc.sync.dma_start(out=xht[:], in_=xhd_d[t])
                    for q in range(4):
                        sn = dec_state("h%d" % q)
                        sl = slice(q * QB, (q + 1) * QB)
                        _gru_step(nc, ps, gp2A, gp2B, QB, wzr["d"],
                                  whh["d"], zrx["d"], oht[:, :, sl],
                                  xht[:, :, sl], hs[q], sn, first=False,
                                  cfg=cfg["dec"])
                        hs[q] = sn
                    if len(pend) >= _PEND_DEPTH[0]:
                        emit_logits(*pend.pop(0))
                    if t == T - 1:
                        emit_logits(dict(hs), t)
                    else:
                        pend.append((dict(hs), t))
                for args in pend:
                    emit_logits(*args)

    nc.compile()
    return nc


def _split8(t):
    hi = np.asarray(t, np.float32).astype(NPF8)
    lo = (np.asarray(t, np.float32) - hi.astype(np.float32)).astype(NPF8)
    return hi, lo


def _ktiles(a, kt, last):
    # [kt*128, last] -> [128, kt, last]
    return np.ascontiguousarray(a.reshape(kt, P, last).transpose(1, 0, 2))


def kernel(**inputs):
    global LAST_RESULT, _CACHED_NC

    sources = np.asarray(inputs["sources"])
    targets = np.asarray(inputs["targets"])
    for k in ("enc_fwd_bx", "enc_fwd_bh", "enc_bwd_bx", "enc_bwd_bh",
              "dec_bx", "dec_bh"):
        if np.any(np.asarray(inputs[k]) != 0):
            raise NotImplementedError(f"nonzero bias {k} not supported")

    shared = {}
    xh_tabs = {}
    for c, whk, wxk, embk in (("f", "enc_fwd_Wh", "enc_fwd_Wx", "src_emb"),
                              ("b", "enc_bwd_Wh", "enc_bwd_Wx", "src_emb"),
                              ("d", "dec_Wh", "dec_Wx", "tgt_emb")):
        wh = np.asarray(inputs[whk], np.float32) * SW
        wx = np.asarray(inputs[wxk], np.float32)
        emb = np.asarray(inputs[embk], np.float32)
        shared["wzr_" + c] = _ktiles(wh[:, :2 * H], KH, 2 * H).astype(NPF8)
        hh_hi, hh_lo = _split8(wh[:, 2 * H:])
        shared["whh_" + c] = np.concatenate(
            [_ktiles(hh_hi, KH, H), _ktiles(hh_lo, KH, H)], axis=1)
        zr_tab = (emb @ wx[:, :2 * H]) * SW            # [V, 2H]
        zr_hi, zr_lo = _split8(zr_tab)
        shared["zrx_" + c] = np.stack([zr_hi, zr_lo], axis=1)  # [V,2,2H]
        xh_tabs[c] = ((emb @ wx[:, 2 * H:]) * SW).astype(np.float16)

    shared["ow"] = _ktiles(np.asarray(inputs["out_W"], np.float32), KH, V
                           ).astype(np.float16)
    shared["outb"] = np.asarray(inputs["out_b"]).reshape(P, 1).astype(np.float32)

    dec_in = np.concatenate(
        [np.full((B, 1), BOW, dtype=targets.dtype), targets[:, :-1]], axis=1)

    vocab = np.arange(V, dtype=np.int32)

    def onehot_tiles(idx):
        # idx [Bc, steps] -> [steps, P, 2, Bc] fp8 (duplicated k-tiles)
        oh = (vocab[None, :, None] == idx.T[:, None, :])  # [steps, V, Bc]
        oh = oh.astype(NPF8)
        return np.ascontiguousarray(
            np.broadcast_to(oh[:, :, None, :],
                            (idx.shape[1], V, 2, idx.shape[0])))

    def xh_tiles(tab, idx):
        # tab [V, H] f16, idx [Bc, steps] -> [steps, P, KH, Bc] f16
        g = tab[idx]                                   # [Bc, steps, H]
        a = g.transpose(1, 2, 0)                       # [steps, H, Bc]
        a = a.reshape(-1, KH, P, a.shape[-1]).transpose(0, 2, 1, 3)
        return np.ascontiguousarray(a)

    in_maps = []
    for cix in range(NCORES):
        sl = slice(cix * BC, (cix + 1) * BC)
        m = dict(shared)
        m["ohe"] = onehot_tiles(sources[sl])
        m["ohd"] = onehot_tiles(dec_in[sl])
        m["xhe"] = xh_tiles(xh_tabs["f"], sources[sl])
        m["xhb"] = xh_tiles(xh_tabs["b"], sources[sl])
        m["xhd"] = xh_tiles(xh_tabs["d"], dec_in[sl])
        in_maps.append(m)

    if _CACHED_NC is None:
        _CACHED_NC = _build_bass()
    nc = _CACHED_NC

    res = run_bass_kernel_spmd(nc, in_maps, core_ids=list(range(NCORES)))
    LAST_RESULT = res

    outs = [np.transpose(r["out"], (2, 0, 1)) for r in res.results]
    return np.ascontiguousarray(np.concatenate(outs, axis=0))


# revision 42
# speedup vs baseline: 1.0082x; 1.0082x over previous
"""Trainium2 Bass kernel for seq2seq GRU — table-lookup fp8 version.

B=4096, S=T=16, V=128, E=256, H=512. Pure data parallel over 8 cores
(batch sharded 512 words/core, weights replicated; forward only, so no
collectives needed).

Key idea: V=128 fits the PE contraction dim, so every x-path matmul
(x @ Wx) collapses into a 128-row table lookup:
  * candidate xh preact: host-gathered EXACT fp16 tiles, DMA'd per step
    (removes the 3-term hi/lo fp8 xh matmuls entirely);
  * z/r gate x-preact: one-hot DoubleRow matmul against an fp8 hi+lo
    table pair (exact to ~1e-3 rel, same PE cost as a direct fp8 x path).

Numerics (rel err ~1.2e-2 vs the 2e-2 budget):
  * state carried fp16 at scale 1; fp8e4m3 'hi' copy feeds the PE;
  * Wh-zr fp8 single; Wh-hh fp8 hi+lo pair at the same x32 scale;
  * out_W fp16; preacts land x32 in PSUM, ACT applies 1/32.

Structure: encoder = 4 independent recurrence chains (2 dirs x 2 batch
halves, W=256); decoder = 4 quarter chains (W=128) + a batched logits
matmul per step. One shared PSUM ring ([P,4,256] x4 = all 8 banks)
spans both phases so no pool-release barrier sits between them.

Engine split per GRU step (tuned against TimelineSim):
  PE    zr-h + one-hot zr-x + hh DoubleRow matmuls (plus dec logits)
  ACT   sigmoid(r), sigmoid(z), tanh(w)
  DVE   v=r*hh (PSUM read), most of w/d/m/T' (fp16 all-SBUF 2x mode),
        dec logit evac
  Pool  small k-tile shares of w/T' (w 1/4 enc, 1/2 dec; m 1/4 dec)
        and the fp16->fp8 hi state copy
"""

import numpy as np
import ml_dtypes

import concourse.bass as bass
import concourse.bacc as bacc
import concourse.mybir as mybir
from concourse.tile import TileContext
from concourse.bass_utils import run_bass_kernel_spmd

F32 = mybir.dt.float32
F16 = mybir.dt.float16
FP8 = mybir.dt.float8e4
AF = mybir.ActivationFunctionType
OP = mybir.AluOpType
DR = mybir.MatmulPerfMode.DoubleRow
NPF8 = ml_dtypes.float8_e4m3fn

P = 128
NCORES = 8
B, S, T = 4096, 16, 16
V, E, H = 128, 256, 512
BC = B // NCORES          # 512 words per core
HB = BC // 2              # encoder chain width
QB = BC // 4              # decoder chain width
KH = H // P               # 4
BOW = 1

SW = 32.0                 # preact scale in PSUM
ISW = float(1.0 / SW)

_ENC_CFG = {"w_kpool": 1, "d_kpool": 0, "m_kpool": 0, "t_kpool": 1,
            "hi_kpool": 4}
_DEC_CFG = {"w_kpool": 2, "d_kpool": 0, "m_kpool": 1, "t_kpool": 0,
            "hi_kpool": 4, "ps_width": HB}
_DEFAULT_CFG = {"enc": _ENC_CFG, "dec": _DEC_CFG, "ohp_bufs": 6,
                "xhp_bufs": 6, "enc_ps_bufs": 4, "dec_ps_bufs": 6,
                "gA_bufs": 3, "gB_bufs": 3, "st_bufs": 2, "std_bufs": 2}

_DEC_W_FETCH_T = [-1]
_LAST_STEP_INTERLEAVE = [False]
_PEND_DEPTH = [1]
_EVAC_SPLIT = [1]

LAST_RESULT = None
_CACHED_NC = None


def _gru_step(nc, ps, gpA, gpB, W, wzr, whh, zrx, oh, xh, st_prev, st_new,
              first, last=False, cfg=None):
    """One GRU step, transposed layout, width W (256 enc / 128 dec).

    oh: one-hot tile slice [P, 2, W] fp8 (k-tiles hi/lo of the zr table);
    xh: exact candidate x-preact slice [P, KH, W] fp16 (x32 scale).
    st_* = {"T16": fp16 state, "hi": fp8 copy}; on the first step only the
    z gate is computed (h=0 -> r unused, h' = (1-z)*c via sigma(-x)).
    """
    PSW = cfg.get("ps_width", W)

    def ptile(nm):
        t = ps.tile([P, KH, PSW], F32, tag="ps", name=nm)
        return t if PSW == W else t[:, :, 0:W]

    if first:
        zt = ptile("z_ps")
        for jj in range(KH):
            col = jj * P
            nc.tensor.matmul(zt[:, jj, :], zrx[:, :, col:col + P], oh,
                             start=True, stop=True, perf_mode=DR)
        zp16 = gpA.tile([P, KH, W], F16, tag="z16", name="zp16")
        nc.scalar.activation(zp16[:], zt[:], AF.Sigmoid, scale=-ISW)
        c16 = gpA.tile([P, KH, W], F16, tag="c16", name="c16")
        nc.scalar.activation(c16[:], xh, AF.Tanh, scale=ISW)
        nc.vector.tensor_tensor(st_new["T16"][:], zp16[:], c16[:], OP.mult)
        if not last:
            nc.gpsimd.tensor_copy(out=st_new["hi"][:], in_=st_new["T16"][:])
        return

    hi = st_prev["hi"]

    def zr_cols(t, jj, col):
        nc.tensor.matmul(t[:, jj, :], wzr[:, 0:2, col:col + P],
                         hi[:, 0:2, :], start=True, stop=False,
                         perf_mode=DR)
        nc.tensor.matmul(t[:, jj, :], wzr[:, 2:4, col:col + P],
                         hi[:, 2:4, :], start=False, stop=False,
                         perf_mode=DR)
        nc.tensor.matmul(t[:, jj, :], zrx[:, :, col:col + P], oh,
                         start=False, stop=True, perf_mode=DR)

    rt = ptile("r_ps")
    for jj in range(KH):
        zr_cols(rt, jj, H + jj * P)
    thh = ptile("hh_ps")
    for jj in range(KH):
        col = jj * P
        nc.tensor.matmul(thh[:, jj, :], whh[:, 0:2, col:col + P],
                         hi[:, 0:2, :], start=True, stop=False, perf_mode=DR)
        nc.tensor.matmul(thh[:, jj, :], whh[:, 2:4, col:col + P],
                         hi[:, 2:4, :], start=False, stop=False, perf_mode=DR)
        nc.tensor.matmul(thh[:, jj, :], whh[:, 4:6, col:col + P],
                         hi[:, 0:2, :], start=False, stop=False, perf_mode=DR)
        nc.tensor.matmul(thh[:, jj, :], whh[:, 6:8, col:col + P],
                         hi[:, 2:4, :], start=False, stop=True, perf_mode=DR)

    zt = ptile("z_ps")
    for jj in range(KH):
        zr_cols(zt, jj, jj * P)
    r16 = gpA.tile([P, KH, W], F16, tag="r16", name="r16")
    nc.scalar.activation(r16[:], rt[:], AF.Sigmoid, scale=ISW)
    z16 = gpA.tile([P, KH, W], F16, tag="z16", name="z16")
    nc.scalar.activation(z16[:], zt[:], AF.Sigmoid, scale=ISW)

    def split_tt(out, a, b, op, kpool):
        # first kpool k-tiles on Pool, rest on DVE
        if kpool > 0:
            nc.gpsimd.tensor_tensor(out[:, 0:kpool, :], a[:, 0:kpool, :],
                                    b[:, 0:kpool, :], op)
        if kpool < KH:
            nc.vector.tensor_tensor(out[:, kpool:KH, :], a[:, kpool:KH, :],
                                    b[:, kpool:KH, :], op)

    v16 = gpB.tile([P, KH, W], F16, tag="v16", name="v16")
    nc.vector.tensor_tensor(v16[:], r16[:], thh[:], OP.mult)
    if cfg.get("w_acc"):
        # accumulate v into the DMA'd xh tile slice in place (4x-mode
        # tensor_scalar with AddAccumulate) -> no separate w tile/op pair
        bi = nc.vector.tensor_scalar(xh, v16[:], 0.0, 1.0,
                                     OP.add, OP.mult)
        bi.ins.acc = "AddAccumulate"
        w16 = xh
    else:
        w16t = gpB.tile([P, KH, W], F16, tag="w16", name="w16")
        split_tt(w16t, v16, xh, OP.add, cfg["w_kpool"])
        w16 = w16t[:]
    c16 = gpA.tile([P, KH, W], F16, tag="c16", name="c16")
    nc.scalar.activation(c16[:], w16, AF.Tanh, scale=ISW)

    d16 = gpB.tile([P, KH, W], F16, tag="d16", name="d16")
    split_tt(d16, st_prev["T16"], c16, OP.subtract, cfg["d_kpool"])
    if cfg.get("acc_blend"):
        # T' = c, then T' += z*d in one accumulating TT (no m tile)
        nc.vector.tensor_copy(out=st_new["T16"][:], in_=c16[:])
        bi = nc.vector.tensor_tensor(st_new["T16"][:], z16[:], d16[:],
                                     OP.mult)
        bi.ins.acc = "AddAccumulate"
    else:
        m16 = gpB.tile([P, KH, W], F16, tag="m16", name="m16")
        split_tt(m16, z16, d16, OP.mult, cfg["m_kpool"])
        split_tt(st_new["T16"], c16, m16, OP.add, cfg["t_kpool"])
    if not last:
        kp = cfg["hi_kpool"]
        if kp > 0:
            nc.gpsimd.tensor_copy(out=st_new["hi"][:, 0:kp, :],
                                  in_=st_new["T16"][:, 0:kp, :])
        if kp < KH:
            nc.vector.tensor_copy(out=st_new["hi"][:, kp:KH, :],
                                  in_=st_new["T16"][:, kp:KH, :])


def _build_bass(cfg=None):
    cfg = dict(_DEFAULT_CFG, **(cfg or {}))
    nc = bacc.Bacc("TRN2", target_bir_lowering=False, debug=False,
                   enable_asserts=False, num_devices=NCORES)

    def din(name, shape, dt=FP8):
        return nc.dram_tensor(name, list(shape), dt, kind="ExternalInput")

    wzr_d = {c: din("wzr_" + c, (P, KH, 2 * H)) for c in "fbd"}
    whh_d = {c: din("whh_" + c, (P, 2 * KH, H)) for c in "fbd"}
    zrx_d = {c: din("zrx_" + c, (P, 2, 2 * H)) for c in "fbd"}
    ohe_d = din("ohe", (S, P, 2, BC))
    ohd_d = din("ohd", (T, P, 2, BC))
    xhe_d = din("xhe", (S, P, KH, BC), F16)
    xhb_d = din("xhb", (S, P, KH, BC), F16)
    xhd_d = din("xhd", (T, P, KH, BC), F16)
    ow_d = din("ow", (P, KH, V), F16)
    outb_d = din("outb", (P, 1), F32)
    out_d = nc.dram_tensor("out", [T, P, BC], F32, kind="ExternalOutput")

    with TileContext(nc) as tc:
        with tc.tile_pool(name="wpool", bufs=1) as wp, \
             tc.tile_pool(name="stenc", bufs=1) as stenc, \
             tc.tile_pool(name="ohp", bufs=cfg["ohp_bufs"]) as ohp, \
             tc.tile_pool(name="xhp", bufs=cfg["xhp_bufs"]) as xhp:

            # ---- weight tiles; DMA order = first-use order ----
            zrx = {c: wp.tile([P, 2, 2 * H], FP8, tag="zrx" + c,
                              name="zrx_" + c) for c in "fbd"}
            wzr = {c: wp.tile([P, KH, 2 * H], FP8, tag="wzr" + c,
                              name="wzr_" + c) for c in "fbd"}
            whh = {c: wp.tile([P, 2 * KH, H], FP8, tag="whh" + c,
                              name="whh_" + c) for c in "fbd"}
            ow_sb = wp.tile([P, KH, V], F16, tag="ow", name="ow_sb")
            outb_sb = wp.tile([P, 1], F32, tag="outb", name="outb_sb")

            # first steps need only the tables + first oh/xh tiles
            pre_oh, pre_xh = {}, {}
            def fetch_step(t, xsrc):
                oht = ohp.tile([P, 2, BC], FP8, tag="oh", name="oh_t")
                nc.sync.dma_start(out=oht[:], in_=ohe_d[t])
                pre_oh[t] = oht
                xht = xhp.tile([P, KH, BC], F16, tag="xh", name="xh_t")
                nc.sync.dma_start(out=xht[:], in_=xsrc[t])
                pre_xh[t] = xht
            nc.sync.dma_start(out=zrx["f"][:], in_=zrx_d["f"][:])
            fetch_step(0, xhe_d)
            nc.sync.dma_start(out=zrx["b"][:], in_=zrx_d["b"][:])
            fetch_step(S - 1, xhb_d)
            for c in "fb":
                nc.sync.dma_start(out=wzr[c][:], in_=wzr_d[c][:])
                nc.sync.dma_start(out=whh[c][:], in_=whh_d[c][:])
            fetch_step(1, xhe_d)
            fetch_step(S - 2, xhb_d)

            def fetch_dec_weights():
                for c in "d":
                    nc.sync.dma_start(out=zrx[c][:], in_=zrx_d[c][:])
                    nc.sync.dma_start(out=wzr[c][:], in_=wzr_d[c][:])
                    nc.sync.dma_start(out=whh[c][:], in_=whh_d[c][:])
                nc.sync.dma_start(out=ow_sb[:], in_=ow_d[:])
                nc.sync.dma_start(out=outb_sb[:], in_=outb_d[:])

            if _DEC_W_FETCH_T[0] < 0:
                fetch_dec_weights()


            # ---- shared pools: one PSUM ring for enc+dec (no release
            # barrier between phases); SBUF pools coexist ----
            with tc.tile_pool(name="ps", bufs=cfg["enc_ps_bufs"], space="PSUM") as ps, \
                 tc.tile_pool(name="st", bufs=cfg["st_bufs"]) as st, \
                 tc.tile_pool(name="gA", bufs=cfg["gA_bufs"]) as gpA, \
                 tc.tile_pool(name="gB", bufs=cfg["gB_bufs"]) as gpB, \
                 tc.tile_pool(name="std", bufs=cfg["std_bufs"]) as std, \
                 tc.tile_pool(name="g2A", bufs=4) as gp2A, \
                 tc.tile_pool(name="g2B", bufs=4) as gp2B, \
                 tc.tile_pool(name="lo", bufs=4) as lop:

                fin = {c + str(hf_): stenc.tile([P, KH, HB], F16,
                                                tag="fin" + c + str(hf_),
                                                name="fin" + c + str(hf_))
                       for c in "fb" for hf_ in range(2)}

                def new_state(tag):
                    return {
                        "T16": st.tile([P, KH, HB], F16, tag=tag + "T",
                                       name=tag + "T"),
                        "hi": st.tile([P, KH, HB], FP8, tag=tag + "h",
                                      name=tag + "h"),
                    }

                stt = {}
                for t in range(S):
                    if t == _DEC_W_FETCH_T[0]:
                        fetch_dec_weights()
                    if t < 2:
                        ohf, ohb = pre_oh[t], pre_oh[S - 1 - t]
                        xhf, xhb = pre_xh[t], pre_xh[S - 1 - t]
                    else:
                        ohf = ohp.tile([P, 2, BC], FP8, tag="oh", name="oh_f")
                        nc.sync.dma_start(out=ohf[:], in_=ohe_d[t])
                        xhf = xhp.tile([P, KH, BC], F16, tag="xh", name="xh_f")
                        nc.sync.dma_start(out=xhf[:], in_=xhe_d[t])
                        ohb = ohp.tile([P, 2, BC], FP8, tag="oh", name="oh_b")
                        nc.sync.dma_start(out=ohb[:], in_=ohe_d[S - 1 - t])
                        xhb = xhp.tile([P, KH, BC], F16, tag="xh", name="xh_b")
                        nc.sync.dma_start(out=xhb[:], in_=xhb_d[S - 1 - t])
                    if t == S - 1 and _LAST_STEP_INTERLEAVE[0]:
                        corder = [("f", ohf, xhf, 0), ("b", ohb, xhb, 0),
                                  ("f", ohf, xhf, 1), ("b", ohb, xhb, 1)]
                    else:
                        corder = [(c, o, x, hf_)
                                  for c, o, x in (("f", ohf, xhf),
                                                  ("b", ohb, xhb))
                                  for hf_ in range(2)]
                    for c, oht, xht, hf_ in corder:
                        if True:
                            key = c + str(hf_)
                            sl = slice(hf_ * HB, (hf_ + 1) * HB)
                            if t == S - 1:
                                sn = {"T16": fin[key], "hi": None}
                            else:
                                sn = new_state(key)
                            _gru_step(nc, ps, gpA, gpB, HB, wzr[c],
                                      whh[c], zrx[c], oht[:, :, sl],
                                      xht[:, :, sl], stt.get(key), sn,
                                      first=(t == 0), last=(t == S - 1),
                                      cfg=cfg["enc"])
                            stt[key] = sn

                # ---- decoder: 4 batch-quarter chains, W = QB ----
                def dec_state(tag):
                    return {
                        "T16": std.tile([P, KH, QB], F16, tag=tag + "T",
                                        name=tag + "T"),
                        "hi": std.tile([P, KH, QB], FP8, tag=tag + "h",
                                       name=tag + "h"),
                    }

                hs = {}
                for q in range(4):
                    s0 = dec_state("i%d" % q)
                    hf_, off = q // 2, (q % 2) * QB
                    nc.vector.tensor_tensor(
                        s0["T16"][:], fin["f%d" % hf_][:, :, off:off + QB],
                        fin["b%d" % hf_][:, :, off:off + QB], OP.add)
                    nc.gpsimd.tensor_copy(out=s0["hi"][:], in_=s0["T16"][:])
                    hs[q] = s0

                def emit_logits(states, t):
                    lpf = ps.tile([P, KH, HB], F32, tag="ps", name="lp")
                    lp = lpf[:, :, 0:QB]
                    for q in range(4):
                        for k in range(KH):
                            nc.tensor.matmul(lp[:, q, :], ow_sb[:, k, :],
                                             states[q]["T16"][:, k, :],
                                             start=(k == 0),
                                             stop=(k == KH - 1))
                    losb = lop.tile([P, 4, QB], F32, tag="lo", name="losb")
                    ke = _EVAC_SPLIT[0]
                    if ke > 0:
                        nc.scalar.activation(losb[:, 0:ke, :], lp[:, 0:ke, :],
                                             AF.Identity, bias=outb_sb[:, 0:1])
                    if ke < 4:
                        nc.vector.tensor_scalar_add(losb[:, ke:4, :],
                                                    lp[:, ke:4, :],
                                                    outb_sb[:, 0:1])
                    nc.sync.dma_start(out=out_d[t], in_=losb[:])

                pend = []
                for t in range(T):
                    oht = ohp.tile([P, 2, BC], FP8, tag="oh", name="ohd_t")
                    nc.sync.dma_start(out=oht[:], in_=ohd_d[t])
                    xht = xhp.tile([P, KH, BC], F16, tag="xh", name="xhd_t")
                    nc.sync.dma_start(out=xht[:], in_=xhd_d[t])
                    for q in range(4):
                        sn = dec_state("h%d" % q)
                        sl = slice(q * QB, (q + 1) * QB)
                        _gru_step(nc, ps, gp2A, gp2B, QB, wzr["d"],
                                  whh["d"], zrx["d"], oht[:, :, sl],
                                  xht[:, :, sl], hs[q], sn, first=False,
                                  cfg=cfg["dec"])
                        hs[q] = sn
                    if len(pend) >= _PEND_DEPTH[0]:
                        emit_logits(*pend.pop(0))
                    if t == T - 1:
                        emit_logits(dict(hs), t)
                    else:
                        pend.append((dict(hs), t))
                for args in pend:
                    emit_logits(*args)

    nc.compile()
    return nc


def _split8(t):
    hi = np.asarray(t, np.float32).astype(NPF8)
    lo = (np.asarray(t, np.float32) - hi.astype(np.float32)).astype(NPF8)
    return hi, lo


def _ktiles(a, kt, last):
    # [kt*128, last] -> [128, kt, last]
    return np.ascontiguousarray(a.reshape(kt, P, last).transpose(1, 0, 2))


def kernel(**inputs):
    global LAST_RESULT, _CACHED_NC

    sources = np.asarray(inputs["sources"])
    targets = np.asarray(inputs["targets"])
    for k in ("enc_fwd_bx", "enc_fwd_bh", "enc_bwd_bx", "enc_bwd_bh",
              "dec_bx", "dec_bh"):
        if np.any(np.asarray(inputs[k]) != 0):
            raise NotImplementedError(f"nonzero bias {k} not supported")

    shared = {}
    xh_tabs = {}
    for c, whk, wxk, embk in (("f", "enc_fwd_Wh", "enc_fwd_Wx", "src_emb"),
                              ("b", "enc_bwd_Wh", "enc_bwd_Wx", "src_emb"),
                              ("d", "dec_Wh", "dec_Wx", "tgt_emb")):
        wh = np.asarray(inputs[whk], np.float32) * SW
        wx = np.asarray(inputs[wxk], np.float32)
        emb = np.asarray(inputs[embk], np.float32)
        shared["wzr_" + c] = _ktiles(wh[:, :2 * H], KH, 2 * H).astype(NPF8)
        hh_hi, hh_lo = _split8(wh[:, 2 * H:])
        shared["whh_" + c] = np.concatenate(
            [_ktiles(hh_hi, KH, H), _ktiles(hh_lo, KH, H)], axis=1)
        zr_tab = (emb @ wx[:, :2 * H]) * SW            # [V, 2H]
        zr_hi, zr_lo = _split8(zr_tab)
        shared["zrx_" + c] = np.stack([zr_hi, zr_lo], axis=1)  # [V,2,2H]
        xh_tabs[c] = ((emb @ wx[:, 2 * H:]) * SW).astype(np.float16)

    shared["ow"] = _ktiles(np.asarray(inputs["out_W"], np.float32), KH, V
                           ).astype(np.float16)
    shared["outb"] = np.asarray(inputs["out_b"]).reshape(P, 1).astype(np.float32)

    dec_in = np.concatenate(
        [np.full((B, 1), BOW, dtype=targets.dtype), targets[:, :-1]], axis=1)

    vocab = np.arange(V, dtype=np.int32)

    def onehot_tiles(idx):
        # idx [Bc, steps] -> [steps, P, 2, Bc] fp8 (duplicated k-tiles)
        oh = (vocab[None, :, None] == idx.T[:, None, :])  # [steps, V, Bc]
        oh = oh.astype(NPF8)
        return np.ascontiguousarray(
            np.broadcast_to(oh[:, :, None, :],
                            (idx.shape[1], V, 2, idx.shape[0])))

    def xh_tiles(tab, idx):
        # tab [V, H] f16, idx [Bc, steps] -> [steps, P, KH, Bc] f16
        g = tab[idx]                                   # [Bc, steps, H]
        a = g.transpose(1, 2, 0)                       # [steps, H, Bc]
        a = a.reshape(-1, KH, P, a.shape[-1]).transpose(0, 2, 1, 3)
        return np.ascontiguousarray(a)

    in_maps = []
    for cix in range(NCORES):
        sl = slice(cix * BC, (cix + 1) * BC)
        m = dict(shared)
        m["ohe"] = onehot_tiles(sources[sl])
        m["ohd"] = onehot_tiles(dec_in[sl])
        m["xhe"] = xh_tiles(xh_tabs["f"], sources[sl])
        m["xhb"] = xh_tiles(xh_tabs["b"], sources[sl])
        m["xhd"] = xh_tiles(xh_tabs["d"], dec_in[sl])
        in_maps.append(m)

    if _CACHED_NC is None:
        _CACHED_NC = _build_bass()
    nc = _CACHED_NC

    res = run_bass_kernel_spmd(nc, in_maps, core_ids=list(range(NCORES)))
    LAST_RESULT = res

    outs = [np.transpose(r["out"], (2, 0, 1)) for r in res.results]
    return np.ascontiguousarray(np.concatenate(outs, axis=0))


# revision 45
# speedup vs baseline: 1.0092x; 1.0010x over previous
"""Trainium2 Bass kernel for seq2seq GRU — table-lookup fp8 version.

B=4096, S=T=16, V=128, E=256, H=512. Pure data parallel over 8 cores
(batch sharded 512 words/core, weights replicated; forward only, so no
collectives needed).

Key idea: V=128 fits the PE contraction dim, so every x-path matmul
(x @ Wx) collapses into a 128-row table lookup:
  * candidate xh preact: host-gathered EXACT fp16 tiles, DMA'd per step
    (removes the 3-term hi/lo fp8 xh matmuls entirely);
  * z/r gate x-preact: one-hot DoubleRow matmul against an fp8 hi+lo
    table pair (exact to ~1e-3 rel, same PE cost as a direct fp8 x path).

Numerics (rel err ~1.2e-2 vs the 2e-2 budget):
  * state carried fp16 at scale 1; fp8e4m3 'hi' copy feeds the PE;
  * Wh-zr fp8 single; Wh-hh fp8 hi+lo pair at the same x32 scale;
  * out_W fp16; preacts land x32 in PSUM, ACT applies 1/32.

Structure: encoder = 4 independent recurrence chains (2 dirs x 2 batch
halves, W=256); decoder = 4 quarter chains (W=128) + a batched logits
matmul per step. One shared PSUM ring ([P,4,256] x4 = all 8 banks)
spans both phases so no pool-release barrier sits between them.

Engine split per GRU step (tuned against TimelineSim):
  PE    zr-h + one-hot zr-x + hh DoubleRow matmuls (plus dec logits)
  ACT   sigmoid(r), sigmoid(z), tanh(w)
  DVE   v=r*hh (PSUM read), most of w/d/m/T' (fp16 all-SBUF 2x mode),
        3/4 of the dec logit evac (1/4 rides ACT, see _EVAC_SPLIT)
  Pool  small k-tile shares of w/T' (w 1/4 enc, 1/2 dec; m 1/4 dec)
        and the fp16->fp8 hi state copy
"""

import numpy as np
import ml_dtypes

import concourse.bass as bass
import concourse.bacc as bacc
import concourse.mybir as mybir
from concourse.tile import TileContext
from concourse.bass_utils import run_bass_kernel_spmd

F32 = mybir.dt.float32
F16 = mybir.dt.float16
FP8 = mybir.dt.float8e4
AF = mybir.ActivationFunctionType
OP = mybir.AluOpType
DR = mybir.MatmulPerfMode.DoubleRow
NPF8 = ml_dtypes.float8_e4m3fn

P = 128
NCORES = 8
B, S, T = 4096, 16, 16
V, E, H = 128, 256, 512
BC = B // NCORES          # 512 words per core
HB = BC // 2              # encoder chain width
QB = BC // 4              # decoder chain width
KH = H // P               # 4
BOW = 1

SW = 32.0                 # preact scale in PSUM
ISW = float(1.0 / SW)

_ENC_CFG = {"w_kpool": 1, "d_kpool": 0, "m_kpool": 0, "t_kpool": 1,
            "hi_kpool": 4}
_DEC_CFG = {"w_kpool": 2, "d_kpool": 0, "m_kpool": 1, "t_kpool": 0,
            "hi_kpool": 4, "ps_width": HB}
_DEFAULT_CFG = {"enc": _ENC_CFG, "dec": _DEC_CFG, "ohp_bufs": 6,
                "xhp_bufs": 6, "enc_ps_bufs": 4, "dec_ps_bufs": 6,
                "gA_bufs": 3, "gB_bufs": 3, "st_bufs": 2, "std_bufs": 2}

_DEC_W_FETCH_T = [-1]
_LAST_STEP_INTERLEAVE = [False]
_PEND_DEPTH = [1]
_EVAC_SPLIT = [1]

LAST_RESULT = None
_CACHED_NC = None


def _gru_step(nc, ps, gpA, gpB, W, wzr, whh, zrx, oh, xh, st_prev, st_new,
              first, last=False, cfg=None):
    """One GRU step, transposed layout, width W (256 enc / 128 dec).

    oh: one-hot tile slice [P, 2, W] fp8 (k-tiles hi/lo of the zr table);
    xh: exact candidate x-preact slice [P, KH, W] fp16 (x32 scale).
    st_* = {"T16": fp16 state, "hi": fp8 copy}; on the first step only the
    z gate is computed (h=0 -> r unused, h' = (1-z)*c via sigma(-x)).
    """
    PSW = cfg.get("ps_width", W)

    def ptile(nm):
        t = ps.tile([P, KH, PSW], F32, tag="ps", name=nm)
        return t if PSW == W else t[:, :, 0:W]

    if first:
        zt = ptile("z_ps")
        for jj in range(KH):
            col = jj * P
            nc.tensor.matmul(zt[:, jj, :], zrx[:, :, col:col + P], oh,
                             start=True, stop=True, perf_mode=DR)
        zp16 = gpA.tile([P, KH, W], F16, tag="z16", name="zp16")
        nc.scalar.activation(zp16[:], zt[:], AF.Sigmoid, scale=-ISW)
        c16 = gpA.tile([P, KH, W], F16, tag="c16", name="c16")
        nc.scalar.activation(c16[:], xh, AF.Tanh, scale=ISW)
        nc.vector.tensor_tensor(st_new["T16"][:], zp16[:], c16[:], OP.mult)
        if not last:
            nc.gpsimd.tensor_copy(out=st_new["hi"][:], in_=st_new["T16"][:])
        return

    hi = st_prev["hi"]

    def zr_cols(t, jj, col):
        nc.tensor.matmul(t[:, jj, :], wzr[:, 0:2, col:col + P],
                         hi[:, 0:2, :], start=True, stop=False,
                         perf_mode=DR)
        nc.tensor.matmul(t[:, jj, :], wzr[:, 2:4, col:col + P],
                         hi[:, 2:4, :], start=False, stop=False,
                         perf_mode=DR)
        nc.tensor.matmul(t[:, jj, :], zrx[:, :, col:col + P], oh,
                         start=False, stop=True, perf_mode=DR)

    rt = ptile("r_ps")
    for jj in range(KH):
        zr_cols(rt, jj, H + jj * P)
    thh = ptile("hh_ps")
    for jj in range(KH):
        col = jj * P
        nc.tensor.matmul(thh[:, jj, :], whh[:, 0:2, col:col + P],
                         hi[:, 0:2, :], start=True, stop=False, perf_mode=DR)
        nc.tensor.matmul(thh[:, jj, :], whh[:, 2:4, col:col + P],
                         hi[:, 2:4, :], start=False, stop=False, perf_mode=DR)
        nc.tensor.matmul(thh[:, jj, :], whh[:, 4:6, col:col + P],
                         hi[:, 0:2, :], start=False, stop=False, perf_mode=DR)
        nc.tensor.matmul(thh[:, jj, :], whh[:, 6:8, col:col + P],
                         hi[:, 2:4, :], start=False, stop=True, perf_mode=DR)

    zt = ptile("z_ps")
    for jj in range(KH):
        zr_cols(zt, jj, jj * P)
    r16 = gpA.tile([P, KH, W], F16, tag="r16", name="r16")
    nc.scalar.activation(r16[:], rt[:], AF.Sigmoid, scale=ISW)
    z16 = gpA.tile([P, KH, W], F16, tag="z16", name="z16")
    nc.scalar.activation(z16[:], zt[:], AF.Sigmoid, scale=ISW)

    def split_tt(out, a, b, op, kpool):
        # first kpool k-tiles on Pool, rest on DVE
        if kpool > 0:
            nc.gpsimd.tensor_tensor(out[:, 0:kpool, :], a[:, 0:kpool, :],
                                    b[:, 0:kpool, :], op)
        if kpool < KH:
            nc.vector.tensor_tensor(out[:, kpool:KH, :], a[:, kpool:KH, :],
                                    b[:, kpool:KH, :], op)

    v16 = gpB.tile([P, KH, W], F16, tag="v16", name="v16")
    nc.vector.tensor_tensor(v16[:], r16[:], thh[:], OP.mult)
    if cfg.get("w_acc"):
        # accumulate v into the DMA'd xh tile slice in place (4x-mode
        # tensor_scalar with AddAccumulate) -> no separate w tile/op pair
        bi = nc.vector.tensor_scalar(xh, v16[:], 0.0, 1.0,
                                     OP.add, OP.mult)
        bi.ins.acc = "AddAccumulate"
        w16 = xh
    else:
        w16t = gpB.tile([P, KH, W], F16, tag="w16", name="w16")
        split_tt(w16t, v16, xh, OP.add, cfg["w_kpool"])
        w16 = w16t[:]
    c16 = gpA.tile([P, KH, W], F16, tag="c16", name="c16")
    if cfg.get("tanh_split"):
        nc.scalar.activation(c16[:, 0:2, :], w16[:, 0:2, :], AF.Tanh,
                             scale=ISW)
        nc.scalar.activation(c16[:, 2:4, :], w16[:, 2:4, :], AF.Tanh,
                             scale=ISW)
    else:
        nc.scalar.activation(c16[:], w16, AF.Tanh, scale=ISW)

    d16 = gpB.tile([P, KH, W], F16, tag="d16", name="d16")
    split_tt(d16, st_prev["T16"], c16, OP.subtract, cfg["d_kpool"])
    if cfg.get("acc_blend"):
        # T' = c, then T' += z*d in one accumulating TT (no m tile)
        nc.vector.tensor_copy(out=st_new["T16"][:], in_=c16[:])
        bi = nc.vector.tensor_tensor(st_new["T16"][:], z16[:], d16[:],
                                     OP.mult)
        bi.ins.acc = "AddAccumulate"
    else:
        m16 = gpB.tile([P, KH, W], F16, tag="m16", name="m16")
        split_tt(m16, z16, d16, OP.mult, cfg["m_kpool"])
        split_tt(st_new["T16"], c16, m16, OP.add, cfg["t_kpool"])
    if not last:
        kp = cfg["hi_kpool"]
        if cfg.get("hi_split") and kp == KH:
            # two aligned pool copies: next-step DR matmuls read k-tiles
            # [0:2] and [2:4] separately, so each half unblocks earlier
            nc.gpsimd.tensor_copy(out=st_new["hi"][:, 0:2, :],
                                  in_=st_new["T16"][:, 0:2, :])
            nc.gpsimd.tensor_copy(out=st_new["hi"][:, 2:4, :],
                                  in_=st_new["T16"][:, 2:4, :])
        else:
            if kp > 0:
                nc.gpsimd.tensor_copy(out=st_new["hi"][:, 0:kp, :],
                                      in_=st_new["T16"][:, 0:kp, :])
            if kp < KH:
                nc.vector.tensor_copy(out=st_new["hi"][:, kp:KH, :],
                                      in_=st_new["T16"][:, kp:KH, :])


def _build_bass(cfg=None):
    cfg = dict(_DEFAULT_CFG, **(cfg or {}))
    nc = bacc.Bacc("TRN2", target_bir_lowering=False, debug=False,
                   enable_asserts=False, num_devices=NCORES)

    def din(name, shape, dt=FP8):
        return nc.dram_tensor(name, list(shape), dt, kind="ExternalInput")

    wzr_d = {c: din("wzr_" + c, (P, KH, 2 * H)) for c in "fbd"}
    whh_d = {c: din("whh_" + c, (P, 2 * KH, H)) for c in "fbd"}
    zrx_d = {c: din("zrx_" + c, (P, 2, 2 * H)) for c in "fbd"}
    ohe_d = din("ohe", (S, P, BC))
    ohd_d = din("ohd", (T, P, BC))
    xhe_d = din("xhe", (S, P, KH, BC), F16)
    xhb_d = din("xhb", (S, P, KH, BC), F16)
    xhd_d = din("xhd", (T, P, KH, BC), F16)
    ow_d = din("ow", (P, KH, V), F16)
    outb_d = din("outb", (P, 1), F32)
    out_d = nc.dram_tensor("out", [T, P, BC], F32, kind="ExternalOutput")

    with TileContext(nc) as tc:
        with tc.tile_pool(name="wpool", bufs=1) as wp, \
             tc.tile_pool(name="stenc", bufs=1) as stenc, \
             tc.tile_pool(name="ohp", bufs=cfg["ohp_bufs"]) as ohp, \
             tc.tile_pool(name="xhp", bufs=cfg["xhp_bufs"]) as xhp:

            # ---- weight tiles; DMA order = first-use order ----
            zrx = {c: wp.tile([P, 2, 2 * H], FP8, tag="zrx" + c,
                              name="zrx_" + c) for c in "fbd"}
            wzr = {c: wp.tile([P, KH, 2 * H], FP8, tag="wzr" + c,
                              name="wzr_" + c) for c in "fbd"}
            whh = {c: wp.tile([P, 2 * KH, H], FP8, tag="whh" + c,
                              name="whh_" + c) for c in "fbd"}
            ow_sb = wp.tile([P, KH, V], F16, tag="ow", name="ow_sb")
            outb_sb = wp.tile([P, 1], F32, tag="outb", name="outb_sb")

            # first steps need only the tables + first oh/xh tiles
            pre_oh, pre_xh = {}, {}
            def fetch_step(t, xsrc):
                oht = ohp.tile([P, BC], FP8, tag="oh", name="oh_t")
                nc.sync.dma_start(out=oht[:], in_=ohe_d[t])
                pre_oh[t] = oht
                xht = xhp.tile([P, KH, BC], F16, tag="xh", name="xh_t")
                nc.sync.dma_start(out=xht[:], in_=xsrc[t])
                pre_xh[t] = xht
            nc.sync.dma_start(out=zrx["f"][:], in_=zrx_d["f"][:])
            fetch_step(0, xhe_d)
            nc.sync.dma_start(out=zrx["b"][:], in_=zrx_d["b"][:])
            fetch_step(S - 1, xhb_d)
            for c in "fb":
                nc.sync.dma_start(out=wzr[c][:], in_=wzr_d[c][:])
                nc.sync.dma_start(out=whh[c][:], in_=whh_d[c][:])
            fetch_step(1, xhe_d)
            fetch_step(S - 2, xhb_d)

            def fetch_dec_weights():
                for c in "d":
                    nc.sync.dma_start(out=zrx[c][:], in_=zrx_d[c][:])
                    nc.sync.dma_start(out=wzr[c][:], in_=wzr_d[c][:])
                    nc.sync.dma_start(out=whh[c][:], in_=whh_d[c][:])
                nc.sync.dma_start(out=ow_sb[:], in_=ow_d[:])
                nc.sync.dma_start(out=outb_sb[:], in_=outb_d[:])

            if _DEC_W_FETCH_T[0] < 0:
                fetch_dec_weights()


            # ---- shared pools: one PSUM ring for enc+dec (no release
            # barrier between phases); SBUF pools coexist ----
            with tc.tile_pool(name="ps", bufs=cfg["enc_ps_bufs"], space="PSUM") as ps, \
                 tc.tile_pool(name="st", bufs=cfg["st_bufs"]) as st, \
                 tc.tile_pool(name="gA", bufs=cfg["gA_bufs"]) as gpA, \
                 tc.tile_pool(name="gB", bufs=cfg["gB_bufs"]) as gpB, \
                 tc.tile_pool(name="std", bufs=cfg["std_bufs"]) as std, \
                 tc.tile_pool(name="g2A", bufs=4) as gp2A, \
                 tc.tile_pool(name="g2B", bufs=4) as gp2B, \
                 tc.tile_pool(name="lo", bufs=4) as lop:

                fin = {c + str(hf_): stenc.tile([P, KH, HB], F16,
                                                tag="fin" + c + str(hf_),
                                                name="fin" + c + str(hf_))
                       for c in "fb" for hf_ in range(2)}

                def new_state(tag):
                    return {
                        "T16": st.tile([P, KH, HB], F16, tag=tag + "T",
                                       name=tag + "T"),
                        "hi": st.tile([P, KH, HB], FP8, tag=tag + "h",
                                      name=tag + "h"),
                    }

                stt = {}
                for t in range(S):
                    if t == _DEC_W_FETCH_T[0]:
                        fetch_dec_weights()
                    if t < 2:
                        ohf, ohb = pre_oh[t], pre_oh[S - 1 - t]
                        xhf, xhb = pre_xh[t], pre_xh[S - 1 - t]
                    else:
                        ohf = ohp.tile([P, BC], FP8, tag="oh", name="oh_f")
                        nc.sync.dma_start(out=ohf[:], in_=ohe_d[t])
                        xhf = xhp.tile([P, KH, BC], F16, tag="xh", name="xh_f")
                        nc.sync.dma_start(out=xhf[:], in_=xhe_d[t])
                        ohb = ohp.tile([P, BC], FP8, tag="oh", name="oh_b")
                        nc.sync.dma_start(out=ohb[:], in_=ohe_d[S - 1 - t])
                        xhb = xhp.tile([P, KH, BC], F16, tag="xh", name="xh_b")
                        nc.sync.dma_start(out=xhb[:], in_=xhb_d[S - 1 - t])
                    if t == S - 1 and _LAST_STEP_INTERLEAVE[0]:
                        corder = [("f", ohf, xhf, 0), ("b", ohb, xhb, 0),
                                  ("f", ohf, xhf, 1), ("b", ohb, xhb, 1)]
                    else:
                        corder = [(c, o, x, hf_)
                                  for c, o, x in (("f", ohf, xhf),
                                                  ("b", ohb, xhb))
                                  for hf_ in range(2)]
                    for c, oht, xht, hf_ in corder:
                        if True:
                            key = c + str(hf_)
                            sl = slice(hf_ * HB, (hf_ + 1) * HB)
                            if t == S - 1:
                                sn = {"T16": fin[key], "hi": None}
                            else:
                                sn = new_state(key)
                            ohb_ap = oht[:, sl].unsqueeze(1) \
                                .broadcast_to((P, 2, HB))
                            _gru_step(nc, ps, gpA, gpB, HB, wzr[c],
                                      whh[c], zrx[c], ohb_ap,
                                      xht[:, :, sl], stt.get(key), sn,
                                      first=(t == 0), last=(t == S - 1),
                                      cfg=cfg["enc"])
                            stt[key] = sn

                # ---- decoder: 4 batch-quarter chains, W = QB ----
                def dec_state(tag):
                    return {
                        "T16": std.tile([P, KH, QB], F16, tag=tag + "T",
                                        name=tag + "T"),
                        "hi": std.tile([P, KH, QB], FP8, tag=tag + "h",
                                       name=tag + "h"),
                    }

                hs = {}
                for q in range(4):
                    s0 = dec_state("i%d" % q)
                    hf_, off = q // 2, (q % 2) * QB
                    nc.vector.tensor_tensor(
                        s0["T16"][:], fin["f%d" % hf_][:, :, off:off + QB],
                        fin["b%d" % hf_][:, :, off:off + QB], OP.add)
                    nc.gpsimd.tensor_copy(out=s0["hi"][:], in_=s0["T16"][:])
                    hs[q] = s0

                def emit_logits(states, t):
                    lpf = ps.tile([P, KH, HB], F32, tag="ps", name="lp")
                    lp = lpf[:, :, 0:QB]
                    for q in range(4):
                        for k in range(KH):
                            nc.tensor.matmul(lp[:, q, :], ow_sb[:, k, :],
                                             states[q]["T16"][:, k, :],
                                             start=(k == 0),
                                             stop=(k == KH - 1))
                    losb = lop.tile([P, 4, QB], F32, tag="lo", name="losb")
                    ke = _EVAC_SPLIT[0]
                    if ke > 0:
                        nc.scalar.activation(losb[:, 0:ke, :], lp[:, 0:ke, :],
                                             AF.Identity, bias=outb_sb[:, 0:1])
                    if ke < 4:
                        nc.vector.tensor_scalar_add(losb[:, ke:4, :],
                                                    lp[:, ke:4, :],
                                                    outb_sb[:, 0:1])
                    nc.sync.dma_start(out=out_d[t], in_=losb[:])

                pend = []
                for t in range(T):
                    oht = ohp.tile([P, BC], FP8, tag="oh", name="ohd_t")
                    nc.sync.dma_start(out=oht[:], in_=ohd_d[t])
                    xht = xhp.tile([P, KH, BC], F16, tag="xh", name="xhd_t")
                    nc.sync.dma_start(out=xht[:], in_=xhd_d[t])
                    for q in range(4):
                        sn = dec_state("h%d" % q)
                        sl = slice(q * QB, (q + 1) * QB)
                        ohq_ap = oht[:, sl].unsqueeze(1) \
                            .broadcast_to((P, 2, QB))
                        _gru_step(nc, ps, gp2A, gp2B, QB, wzr["d"],
                                  whh["d"], zrx["d"], ohq_ap,
                                  xht[:, :, sl], hs[q], sn, first=False,
                                  cfg=cfg["dec"])
                        hs[q] = sn
                    if len(pend) >= _PEND_DEPTH[0]:
                        emit_logits(*pend.pop(0))
                    if t == T - 1:
                        emit_logits(dict(hs), t)
                    else:
                        pend.append((dict(hs), t))
                for args in pend:
                    emit_logits(*args)

    nc.compile()
    return nc


def _split8(t):
    hi = np.asarray(t, np.float32).astype(NPF8)
    lo = (np.asarray(t, np.float32) - hi.astype(np.float32)).astype(NPF8)
    return hi, lo


def _ktiles(a, kt, last):
    # [kt*128, last] -> [128, kt, last]
    return np.ascontiguousarray(a.reshape(kt, P, last).transpose(1, 0, 2))


def kernel(**inputs):
    global LAST_RESULT, _CACHED_NC

    sources = np.asarray(inputs["sources"])
    targets = np.asarray(inputs["targets"])
    for k in ("enc_fwd_bx", "enc_fwd_bh", "enc_bwd_bx", "enc_bwd_bh",
              "dec_bx", "dec_bh"):
        if np.any(np.asarray(inputs[k]) != 0):
            raise NotImplementedError(f"nonzero bias {k} not supported")

    shared = {}
    xh_tabs = {}
    for c, whk, wxk, embk in (("f", "enc_fwd_Wh", "enc_fwd_Wx", "src_emb"),
                              ("b", "enc_bwd_Wh", "enc_bwd_Wx", "src_emb"),
                              ("d", "dec_Wh", "dec_Wx", "tgt_emb")):
        wh = np.asarray(inputs[whk], np.float32) * SW
        wx = np.asarray(inputs[wxk], np.float32)
        emb = np.asarray(inputs[embk], np.float32)
        shared["wzr_" + c] = _ktiles(wh[:, :2 * H], KH, 2 * H).astype(NPF8)
        hh_hi, hh_lo = _split8(wh[:, 2 * H:])
        shared["whh_" + c] = np.concatenate(
            [_ktiles(hh_hi, KH, H), _ktiles(hh_lo, KH, H)], axis=1)
        zr_tab = (emb @ wx[:, :2 * H]) * SW            # [V, 2H]
        zr_hi, zr_lo = _split8(zr_tab)
        shared["zrx_" + c] = np.stack([zr_hi, zr_lo], axis=1)  # [V,2,2H]
        xh_tabs[c] = ((emb @ wx[:, 2 * H:]) * SW).astype(np.float16)

    shared["ow"] = _ktiles(np.asarray(inputs["out_W"], np.float32), KH, V
                           ).astype(np.float16)
    shared["outb"] = np.asarray(inputs["out_b"]).reshape(P, 1).astype(np.float32)

    dec_in = np.concatenate(
        [np.full((B, 1), BOW, dtype=targets.dtype), targets[:, :-1]], axis=1)

    vocab = np.arange(V, dtype=np.int32)

    def onehot_tiles(idx):
        # idx [Bc, steps] -> [steps, P, Bc] fp8 (k-tile pair is read via a
        # stride-0 broadcast AP on device)
        oh = (vocab[None, :, None] == idx.T[:, None, :])  # [steps, V, Bc]
        return np.ascontiguousarray(oh.astype(NPF8))

    def xh_tiles(tab, idx):
        # tab [V, H] f16, idx [Bc, steps] -> [steps, P, KH, Bc] f16
        g = tab[idx]                                   # [Bc, steps, H]
        a = g.transpose(1, 2, 0)                       # [steps, H, Bc]
        a = a.reshape(-1, KH, P, a.shape[-1]).transpose(0, 2, 1, 3)
        return np.ascontiguousarray(a)

    in_maps = []
    for cix in range(NCORES):
        sl = slice(cix * BC, (cix + 1) * BC)
        m = dict(shared)
        m["ohe"] = onehot_tiles(sources[sl])
        m["ohd"] = onehot_tiles(dec_in[sl])
        m["xhe"] = xh_tiles(xh_tabs["f"], sources[sl])
        m["xhb"] = xh_tiles(xh_tabs["b"], sources[sl])
        m["xhd"] = xh_tiles(xh_tabs["d"], dec_in[sl])
        in_maps.append(m)

    if _CACHED_NC is None:
        _CACHED_NC = _build_bass()
    nc = _CACHED_NC

    res = run_bass_kernel_spmd(nc, in_maps, core_ids=list(range(NCORES)))
    LAST_RESULT = res

    outs = [np.transpose(r["out"], (2, 0, 1)) for r in res.results]
    return np.ascontiguousarray(np.concatenate(outs, axis=0))
